# revision 44
# baseline (speedup 1.0000x reference)
"""Trainium2 Bass kernel for nn_BiARMA (2-layer ARMA GNN, K=2 stacks, T=2).

Math: A = D^-1/2 C D^-1/2 (C = edge-count matrix, deg by dst).
Key identity: norm[e] = dinv[src]*dinv[dst] factors, so
  segment_sum(out[src]*norm, dst) = dinv_dst * gather_sum(dinv_src*out[src])
-> every message-passing round is a pure row-gather-accumulate of a
pre-scaled node tensor.  Weights commute with aggregation, so matmuls
run on the aggregated tensor.

Distribution: dst-nodes sharded over 8 cores (graph parallel).  Each
core keeps a full replica of the current pre-scaled node tensor in its
DRAM, gathers rows for its local edges with the gpsimd dma_gather
ucode op, reduces padded per-node slots on DVE, applies weights on PE,
and AllGathers its updated shard each round.

Gather indices are int16 (<=32768 addressable rows), but the replica
has 50176 rows.  Two gather calls per group with OVERLAPPING windows:
call A reads rows [0, 32768) and call B reads rows [17408, 50176) of
the same replica.  Sources on cores 0-2 must use call A, cores 5-7
call B, and cores 3-4 (filled with the highest out-degree nodes) can
use either -- those flexible edges are assigned per destination to
balance the two calls, which makes the per-block padded slot counts
(max over cores x 128 partition rows) nearly tight: ~899 padded slot
columns vs 782 ideal vs 1387 for a disjoint-half split.
"""

import os
import sys
from dataclasses import dataclass, field

import numpy as np

sys.path.insert(0, "/opt/trn_rl_repo")

P = 128
WIN = 32768  # rows addressable by one int16-indexed gather call


@dataclass
class Cfg:
    N: int = 50000
    E: int = 800000
    IN_C: int = 64
    HID_C: int = 64
    OUT_C: int = 32
    K: int = 2
    CORES: int = 8
    # gather-tile budget, elements per partition per group (per dtype)
    group_budget_elems: int = int(os.environ.get("GNN_BUDGET", "6144"))

    @property
    def blocks(self):
        return (self.N // self.CORES + 1 + P - 1) // P

    @property
    def NPC(self):
        return self.blocks * P

    @property
    def NREP(self):
        return self.CORES * self.NPC

    @property
    def OFFB(self):  # window-B base row
        return self.NREP - WIN


@dataclass
class Struct:
    DA: list
    DB: list
    col_off: list       # per-block column offset (A+B combined)
    a_off: list         # per-block offset into the A column space
    b_off: list         # per-block offset into the B column space
    tot_cols: int
    idx16: np.ndarray   # [CORES, 128, (TA+TB)*8] int16 wrapped+replicated
    idx32: np.ndarray   # [CORES, P, tot_cols] int32, -1 padded (deg helper)
    pid: np.ndarray
    a_cum: list = None
    b_cum: list = None
    TA: int = 0
    TB: int = 0
    groups: dict = field(default_factory=dict)


def build_structure(edge_index: np.ndarray, cfg: Cfg) -> Struct:
    src = np.asarray(edge_index[0], dtype=np.int64)
    dst = np.asarray(edge_index[1], dtype=np.int64)
    N, CORES, NPC, NB = cfg.N, cfg.CORES, cfg.NPC, cfg.blocks
    OFFB = cfg.OFFB

    # ---- core assignment: highest out-degree nodes fill cores 3,4 (the
    # cores whose pid range lies inside BOTH gather windows), everything
    # else round-robins over the remaining cores ----
    outdeg = np.bincount(src, minlength=N)
    o = np.argsort(-outdeg, kind="stable")
    core_of = np.empty(N, np.int64)
    nflex = 2 * NPC
    core_of[o[:nflex]] = np.where(np.arange(nflex) % 2 == 0, 3, 4)
    rest_cores = np.array([0, 1, 2, 5, 6, 7])
    core_of[o[nflex:]] = rest_cores[np.arange(N - nflex) % 6]

    # ---- call assignment per edge: by source core, flexible edges
    # (src on cores 3,4) greedily balance each destination's two calls ----
    sc = core_of[src]
    a_fixed = sc <= 2
    b_fixed = sc >= 5
    cA = np.zeros(N, np.int64)
    cB = np.zeros(N, np.int64)
    np.add.at(cA, dst[a_fixed], 1)
    np.add.at(cB, dst[b_fixed], 1)
    half = np.where(a_fixed, 0, 1).astype(np.int64)
    fidx = np.nonzero(~a_fixed & ~b_fixed)[0]
    fidx = fidx[np.argsort(dst[fidx], kind="stable")]
    fd = dst[fidx]
    # per-dst greedy balance, vectorized: within each dst's flexible run,
    # first |imb| edges go to the lighter side, then alternate
    runs = np.concatenate([[0], np.cumsum(np.bincount(fd, minlength=N))])
    pos = np.arange(fidx.shape[0]) - runs[fd]
    imb = (cA - cB)[fd]  # >0: A heavier -> first flex edges go to B
    rem = pos - np.abs(imb)
    go_b = np.where(rem < 0, imb > 0, (rem % 2 == 1) ^ (imb < 0))
    half[fidx] = go_b.astype(np.int64)
    cA = np.zeros(N, np.int64)
    cB = np.zeros(N, np.int64)
    np.add.at(cA, dst[half == 0], 1)
    np.add.at(cB, dst[half == 1], 1)

    # ---- local ordering within each core: lexicographic (cA, cB) desc
    # -> tight per-block padded widths ----
    local_of = np.empty(N, np.int64)
    for c in range(CORES):
        nodes = np.nonzero(core_of == c)[0]
        o2 = np.lexsort((-cB[nodes], -cA[nodes]))
        local_of[nodes[o2]] = np.arange(len(nodes))
    pid = core_of * NPC + local_of

    # window sanity: A-call sources have pid < WIN, B-call >= OFFB
    spid = pid[src]
    assert spid[half == 0].max() < WIN
    assert spid[half == 1].min() >= OFFB

    ecore = core_of[dst]
    dloc = local_of[dst]
    sloc = np.where(half == 0, spid, spid - OFFB)  # window-local index

    # per (call, core, node) counts -> per-block padded A/B widths
    cnt = np.zeros((2, CORES, NPC), np.int64)
    for h in (0, 1):
        for c in range(CORES):
            m = (ecore == c) & (half == h)
            cnt[h, c] = np.bincount(dloc[m], minlength=NPC)
    DA = cnt[0].reshape(CORES, NB, P).max(axis=(0, 2))
    DB = cnt[1].reshape(CORES, NB, P).max(axis=(0, 2))
    DA = np.maximum(DA, 1).tolist()
    DB = np.maximum(DB, 1).tolist()
    D = [DA[b] + DB[b] for b in range(NB)]
    col_off = np.concatenate([[0], np.cumsum(D)]).tolist()
    a_off = [col_off[b] for b in range(NB)]          # A slots first per block
    b_off = [col_off[b] + DA[b] for b in range(NB)]  # then B slots
    tot_cols = int(col_off[-1])

    # per-slot values, node-major layout [P, tot_cols]
    vals = np.full((CORES, P, tot_cols), -1, np.int64)
    eo = np.lexsort((dloc, ecore))
    ecore_s, dloc_s, sloc_s, half_s = ecore[eo], dloc[eo], sloc[eo], half[eo]
    aoff = np.asarray(a_off)
    boff = np.asarray(b_off)
    for c in range(CORES):
        m = ecore_s == c
        dl, sl, hh = dloc_s[m], sloc_s[m], half_s[m]
        for h in (0, 1):
            mh = hh == h
            dlh, slh = dl[mh], sl[mh]
            cth = np.bincount(dlh, minlength=NPC)
            starts = np.concatenate([[0], np.cumsum(cth)])[:-1]
            p_in = np.arange(dlh.shape[0]) - starts[dlh]
            b = dlh // P
            p = dlh % P
            col = (aoff if h == 0 else boff)[b] + p_in
            vals[c, p, col] = slh

    # int32 deg helper (-1 = pad)
    idx32 = vals.astype(np.int32)

    st = Struct(DA=DA, DB=DB, col_off=col_off, a_off=a_off, b_off=b_off,
                tot_cols=tot_cols, idx16=None, idx32=idx32, pid=pid)

    def make_groups(budget_cols):
        budget = max(budget_cols, max(D))
        groups = []
        b0 = 0
        while b0 < NB:
            b1 = b0
            tot = 0
            while b1 < NB and (tot + D[b1] <= budget or b1 == b0):
                tot += D[b1]
                b1 += 1
            groups.append((b0, b1))
            b0 = b1
        return groups

    # every round gathers 128-bf16 rows, so group budget is per-round
    # uniform (elements per partition per group / 128 cols)
    be = cfg.group_budget_elems
    st.groups = {r: make_groups(be // 128) for r in (1, 2, 3, 4)}

    # int16 gather arrays.  For ANY contiguous block range, call A reads
    # the A-columns of blocks b0..b1 in block order, call B the
    # B-columns.  Store two wrapped arrays (all A-columns block-major,
    # then all B-columns) so every call's index slice is contiguous.
    # Flat call position i -> (partition i%128, out column i//128);
    # wrapped storage (i%16, i//16), replicated x8 to 128 partitions.
    a_cum = np.concatenate([[0], np.cumsum(DA)]).astype(np.int64)
    b_cum = np.concatenate([[0], np.cumsum(DB)]).astype(np.int64)
    TA, TB = int(a_cum[-1]), int(b_cum[-1])
    st.a_cum = a_cum.tolist()
    st.b_cum = b_cum.tolist()
    st.TA, st.TB = TA, TB

    # pad rows: phantom (always-zero) rows inside each window
    PADA = NPC - 1               # core 0, top local -- phantom
    PADB = CORES * NPC - 1 - OFFB  # core 7, top local, window-B-relative
    assert PADA < WIN and 0 <= PADB < WIN

    idx16 = np.empty((CORES, 16, (TA + TB) * 8), np.int16)
    for c in range(CORES):
        va = np.empty((P, TA), np.int64)
        vb = np.empty((P, TB), np.int64)
        for b in range(NB):
            va[:, a_cum[b]:a_cum[b + 1]] = \
                vals[c][:, a_off[b]:a_off[b] + DA[b]]
            vb[:, b_cum[b]:b_cum[b + 1]] = \
                vals[c][:, b_off[b]:b_off[b] + DB[b]]
        va = np.where(va < 0, PADA, va)
        vb = np.where(vb < 0, PADB, vb)
        both = np.concatenate([va, vb], axis=1).astype(np.int16)
        # column c', partition p -> flat i = c'*128 + p -> (i%16, i//16):
        # wrapped[r, c'*8 + q] with p = q*16 + r
        w = both.reshape(16, 8, TA + TB, order="F")
        w2 = np.transpose(w, (0, 2, 1)).reshape(16, (TA + TB) * 8)
        idx16[c] = w2
    st.idx16 = np.tile(idx16, (1, 8, 1))
    return st


# packed-weight column layout in the single [128, 768] f32 input:
# w1bd 0:128 | rootw1 128:256 | initw1 256:384 | rootw2 384:448 |
# initw2 448:512 | w2bd 512:576 | b1 576:704 (row 0) | b2 704:768 (row 0)
WPACK_COLS = 768


def build_weight_inputs(inp: dict, cfg: Cfg) -> dict:
    K, IN_C, HID_C, OUT_C = cfg.K, cfg.IN_C, cfg.HID_C, cfg.OUT_C

    rootw1 = np.transpose(inp["root_w1"][0], (1, 0, 2)).reshape(IN_C, K * HID_C)
    b1row = inp["b1"][0, :, 0, :].reshape(1, K * HID_C)
    initw1 = np.transpose(inp["init_w1"], (1, 0, 2)).reshape(IN_C, K * HID_C)
    w1bd = np.zeros((K * HID_C, K * HID_C), np.float32)
    for k in range(K):
        w1bd[k * HID_C:(k + 1) * HID_C, k * HID_C:(k + 1) * HID_C] = inp["w1"][0, k]

    # 0.5 absorbed: round-2 h-stage feeds the UNhalved stack sum into root2
    rootw2 = 0.5 * np.transpose(inp["root_w2"][0], (1, 0, 2)).reshape(HID_C, K * OUT_C)
    b2row = inp["b2"][0, :, 0, :].reshape(1, K * OUT_C)
    initw2 = np.transpose(inp["init_w2"], (1, 0, 2)).reshape(HID_C, K * OUT_C)
    w2bd = np.zeros((K * OUT_C, K * OUT_C), np.float32)
    for k in range(K):
        w2bd[k * OUT_C:(k + 1) * OUT_C, k * OUT_C:(k + 1) * OUT_C] = inp["w2"][0, k]

    wp = np.zeros((128, WPACK_COLS), np.float32)
    wp[:, 0:128] = w1bd
    wp[:IN_C, 128:256] = rootw1
    wp[:IN_C, 256:384] = initw1
    wp[:HID_C, 384:448] = rootw2
    wp[:HID_C, 448:512] = initw2
    wp[:K * OUT_C, 512:576] = w2bd
    wp[0, 576:704] = b1row[0]
    wp[0, 704:768] = b2row[0]
    return {"wpack": wp}


def build_nc(cfg: Cfg, st: Struct):
    import concourse.bacc as bacc
    import concourse.bass as bass
    import concourse.mybir as mybir
    import concourse.tile as tile
    from concourse.masks import make_identity

    f32 = mybir.dt.float32
    bf16 = mybir.dt.bfloat16
    i16 = mybir.dt.int16
    i32 = mybir.dt.int32
    X = mybir.AxisListType.X
    Alu = mybir.AluOpType
    Act = mybir.ActivationFunctionType

    K, IN_C, HID_C, OUT_C = cfg.K, cfg.IN_C, cfg.HID_C, cfg.OUT_C
    G1 = K * HID_C   # 128
    G2 = K * OUT_C   # 64
    NB = cfg.blocks
    NPC, NREP, OFFB = cfg.NPC, cfg.NREP, cfg.OFFB
    DA, DB = st.DA, st.DB
    TA, TB = st.TA, st.TB
    a_cum, b_cum = st.a_cum, st.b_cum
    WTOT = (TA + TB) * 8

    NQ = int(os.environ.get("GNN_QUEUES", "4"))
    nc = bacc.Bacc(
        "TRN2",
        target_bir_lowering=False,
        debug=False,
        num_devices=cfg.CORES,
        num_swdge_queues=NQ,
    )

    # ---- kernel I/O ----
    xs = nc.dram_tensor("xs", [NPC, IN_C], f32, kind="ExternalInput")
    idx16_d = nc.dram_tensor("idx16", [P, WTOT], i16, kind="ExternalInput")
    dinv_d = nc.dram_tensor("dinv", [P, NB], f32, kind="ExternalInput")
    wpack_d = nc.dram_tensor("wpack", [P, WPACK_COLS], f32, kind="ExternalInput")
    out_d = nc.dram_tensor("out", [NPC, OUT_C], f32, kind="ExternalOutput")

    # ---- internal DRAM: every replica row is 128 bf16 = 256B (the
    # gather's minimum descriptor payload).  Rounds whose real feature
    # width is 64 use only the lower half of each row; the upper half is
    # whatever garbage the writers leave there -- it is gathered but the
    # reduce never reads it.  bf16/elem-128 gathers measure ~2.2x faster
    # than f32/elem-64 at the same descriptor count and bytes. ----
    FG = 128  # gathered row width (bf16 elems) for every round
    y = {
        r: nc.dram_tensor(f"y{r}", [NREP, FG], bf16, addr_space="Shared")
        for r in (1, 2, 3, 4)
    }
    # gather-side mirrors in plain DRAM: random 256B reads from Shared
    # address space measure ~165us/round slower than from local DRAM, so
    # after each AllGather the replica is copied (sequential, ~50us) and
    # the gathers read the mirror.
    ym = {
        r: nc.dram_tensor(f"ym{r}", [NREP, FG], bf16)
        for r in (1, 2, 3, 4)
    }
    ag_in = {
        r: nc.dram_tensor(f"agin{r}", [NPC, FG], bf16)
        for r in (1, 2, 3, 4)
    }
    # consumed feature width per round (lower slice of each gathered row)
    FW = {1: IN_C, 2: G1, 3: HID_C, 4: G2}
    GW = {1: G1, 2: G1, 3: G2, 4: G2}
    if os.environ.get("GNN_LITE") == "5":
        ydummy = nc.dram_tensor("ydummy", [NREP, FG], bf16)

    rg = [list(range(cfg.CORES))]

    max_gt = max(
        max((st.col_off[b1] - st.col_off[b0]) * FG
            for (b0, b1) in st.groups[r])
        for r in (1, 2, 3, 4)
    )

    NSEM = 8
    dsems = [nc.alloc_semaphore(f"gsem{i}") for i in range(NSEM)]
    sem_count = [0] * NSEM
    gidx = [0]  # global gather-call counter
    wait_a = [None]
    wait_b = [None]

    with tile.TileContext(nc) as tc:
        with (
            tc.tile_pool(name="const", bufs=1) as cpool,
            tc.tile_pool(name="gather", bufs=int(os.environ.get("GNN_GBUFS", "5"))) as gpool,
            tc.tile_pool(name="work", bufs=3) as wpool,
            tc.tile_pool(name="psum", bufs=3, space="PSUM") as ppool,
        ):
            # ---------- constants ----------
            ident = cpool.tile([P, P], f32)
            make_identity(nc, ident[:])

            wsb = cpool.tile([P, WPACK_COLS], f32)
            nc.sync.dma_start(out=wsb[:], in_=wpack_d[:, :])
            w1bd_s = wsb[:, 0:128]
            rootw1_s = wsb[0:IN_C, 128:256]
            initw1_s = wsb[0:IN_C, 256:384]
            rootw2_s = wsb[0:HID_C, 384:448]
            initw2_s = wsb[0:HID_C, 448:512]
            w2bd_s = wsb[0:G2, 512:576]
            b1_s = wsb[0:1, 576:704]
            b2_s = wsb[0:1, 704:768]
            rhs_s = {1: initw1_s, 2: w1bd_s, 3: initw2_s, 4: w2bd_s}

            ones1 = cpool.tile([1, P], f32)
            nc.vector.memset(ones1[:], 1.0)
            b1rep = cpool.tile([P, G1], f32)
            b2rep = cpool.tile([P, G2], f32)
            bps = ppool.tile([P, G1], f32, tag="mmps")
            nc.tensor.matmul(bps[:], lhsT=ones1[:], rhs=b1_s, start=True, stop=True)
            nc.vector.tensor_copy(b1rep[:], bps[:])
            bps2 = ppool.tile([P, G2], f32, tag="mmps")
            nc.tensor.matmul(bps2[:], lhsT=ones1[:], rhs=b2_s, start=True, stop=True)
            nc.vector.tensor_copy(b2rep[:], bps2[:])

            # ---------- gather indices + degrees ----------
            idx16_s = cpool.tile([P, WTOT], i16)
            nc.sync.dma_start(out=idx16_s[:], in_=idx16_d[:, :])

            root1 = cpool.tile([P, NB, G1], f32)
            root2 = cpool.tile([P, NB, G2], f32)
            dinv = cpool.tile([P, NB], f32)
            dinvh = cpool.tile([P, NB], f32)
            nc.sync.dma_start(out=dinv[:], in_=dinv_d[:, :])
            nc.vector.tensor_scalar_mul(dinvh[:], dinv[:], 0.5)
            with tc.tile_pool(name="prolog", bufs=1) as qpool:
                # ---------- Y1 first (the AllGather only needs the
                # dinv-scaled x), THEN the collective, THEN the root
                # matmuls -- roots overlap AG1 and round 1's gathers ----
                x_s = qpool.tile([P, NB, IN_C], f32)
                for b in range(NB):
                    nc.sync.dma_start(
                        out=x_s[:, b, :], in_=xs[b * P:(b + 1) * P, :]
                    )
                for b in range(NB):
                    dcol = dinv[:, b:b + 1]
                    y1b = wpool.tile([P, 128], bf16, tag="yout16")
                    nc.scalar.activation(
                        y1b[:, :IN_C], x_s[:, b, :], Act.Copy, scale=dcol
                    )
                    nc.sync.dma_start(
                        out=ag_in[1][b * P:(b + 1) * P, :], in_=y1b[:]
                    )
                if not os.environ.get("GNN_SKIP_AG0"):
                    nc.gpsimd.collective_compute(
                        "AllGather", Alu.bypass, replica_groups=rg,
                        ins=[ag_in[1].ap().opt()], outs=[y[1].ap().opt()],
                    )
                    nc.sync.dma_start(out=ym[1][:, :], in_=y[1][:, :])
                for b in range(NB):
                    xT_ps = ppool.tile([IN_C, P], f32, tag="tps")
                    nc.tensor.transpose(xT_ps[:], x_s[:, b, :], ident[:])
                    xT = wpool.tile([IN_C, P], f32, tag="aggT")
                    nc.scalar.activation(xT[:], xT_ps[:], Act.Copy)
                    r1_ps = ppool.tile([P, G1], f32, tag="mmps")
                    nc.tensor.matmul(
                        r1_ps[:], lhsT=xT[:], rhs=rootw1_s, start=True, stop=True
                    )
                    nc.vector.tensor_add(root1[:, b, :], r1_ps[:], b1rep[:])

            # ---------- 4 message-passing rounds ----------
            max_round = int(os.environ.get("GNN_STAGE", "4"))
            reps = int(os.environ.get("GNN_REPS", "1"))

            # first groups of each round read the Shared tensor directly
            # (ready right after the AllGather) so the mirror copy
            # overlaps them instead of delaying the round start
            YDIRECT = int(os.environ.get("GNN_YDIRECT", "5"))

            def gather_group(r, b0, b1, gi=99):
                """Issue the A and B dma_gather calls for blocks [b0,b1);
                returns the gather tile (cols: A of b0..b1, then B)."""
                nA = (a_cum[b1] - a_cum[b0]) * P
                nB = (b_cum[b1] - b_cum[b0]) * P
                ncols = (nA + nB) // P
                gt = gpool.tile([P, max_gt], bf16, tag="gt")
                outA = gt[:, :nA // P * FG].rearrange("p (c f) -> p c f", f=FG)
                outB = gt[:, nA // P * FG:ncols * FG].rearrange(
                    "p (c f) -> p c f", f=FG
                )
                ixA = idx16_s[:, a_cum[b0] * 8:a_cum[b1] * 8]
                ixB = idx16_s[:, (TA + b_cum[b0]) * 8:(TA + b_cum[b1]) * 8]
                if os.environ.get("GNN_LITE") == "5":
                    yv = ydummy
                else:
                    yv = y[r] if gi < YDIRECT else ym[r]
                sA = (2 * gidx[0]) % NSEM
                sB = (2 * gidx[0] + 1) % NSEM
                qA = (2 * gidx[0]) % NQ
                qB = (2 * gidx[0] + 1) % NQ
                gidx[0] += 1
                sem_count[sA] += 16
                sem_count[sB] += 16
                nc.gpsimd.dma_gather(
                    out_ap=outA, in_ap=yv[0:WIN, :], idxs_ap=ixA,
                    num_idxs=nA, num_idxs_reg=nA, elem_size=FG,
                    single_packet=False, queue_num=qA,
                ).then_inc(dsems[sA], 16)
                nc.gpsimd.dma_gather(
                    out_ap=outB, in_ap=yv[OFFB:NREP, :], idxs_ap=ixB,
                    num_idxs=nB, num_idxs_reg=nB, elem_size=FG,
                    single_packet=False, queue_num=qB,
                ).then_inc(dsems[sB], 16)
                wait_a[0] = (dsems[sA], sem_count[sA])
                wait_b[0] = (dsems[sB], sem_count[sB])
                return gt, nA // P

            lite = int(os.environ.get("GNN_LITE", "0"))
            for rep in range(reps):
              for r in (1, 2, 3, 4):
                if r > max_round:
                    break
                F = FW[r]
                G = GW[r]
                for gi, (b0, b1) in enumerate(st.groups[r]):
                    gt, colsA = gather_group(r, b0, b1, gi)
                    if lite == 6:
                        # microbench-style: Pool-side flow control only
                        if gidx[0] > 3:
                            gpast = gidx[0] - 4
                            sa = (2 * gpast) % NSEM
                            nc.gpsimd.wait_ge(
                                dsems[sa], 16 * (gpast // (NSEM // 2) + 1)
                            )
                        continue
                    if lite in (1, 4, 5):
                        lt = wpool.tile([P, 2 * FG], f32, tag="lt")
                        cpa = nc.vector.tensor_copy(lt[:, :FG], gt[:, :FG])
                        cpb = nc.vector.tensor_copy(
                            lt[:, FG:], gt[:, colsA * FG:(colsA + 1) * FG]
                        )
                        if lite == 1:
                            cpa._wait_ge(*wait_a[0])
                            cpb._wait_ge(*wait_b[0])
                        continue
                    for b in range(b0, b1):
                        dcol = dinv[:, b:b + 1]
                        oA = a_cum[b] - a_cum[b0]
                        oB = colsA + (b_cum[b] - b_cum[b0])
                        aggA = wpool.tile([P, F], f32, tag="aggA")
                        rA = nc.vector.reduce_sum(
                            aggA[:],
                            gt[:, oA * FG:(oA + DA[b]) * FG].rearrange(
                                "p (d f) -> p f d", f=FG
                            )[:, 0:F, :],
                            axis=X,
                        )
                        aggB = wpool.tile([P, F], f32, tag="aggB")
                        rB = nc.vector.reduce_sum(
                            aggB[:],
                            gt[:, oB * FG:(oB + DB[b]) * FG].rearrange(
                                "p (d f) -> p f d", f=FG
                            )[:, 0:F, :],
                            axis=X,
                        )
                        rA._wait_ge(*wait_a[0])
                        rB._wait_ge(*wait_b[0])
                        if lite == 2:
                            continue
                        agg = wpool.tile([P, F], f32, tag="agg")
                        nc.vector.tensor_add(agg[:], aggA[:], aggB[:])
                        aggT_ps = ppool.tile([F, P], f32, tag="tps")
                        nc.tensor.transpose(aggT_ps[:], agg[:], ident[:])
                        aggT = wpool.tile([F, P], f32, tag="aggT")
                        nc.scalar.activation(aggT[:], aggT_ps[:], Act.Copy)
                        mm_ps = ppool.tile([P, G], f32, tag="mmps")
                        nc.tensor.matmul(
                            mm_ps[:], lhsT=aggT[:], rhs=rhs_s[r],
                            start=True, stop=True,
                        )
                        if lite == 3:
                            continue
                        root = root1 if r <= 2 else root2
                        t_sb = wpool.tile([P, G], f32, tag="tsb")
                        nc.vector.scalar_tensor_tensor(
                            t_sb[:], mm_ps[:], dcol, root[:, b, :],
                            op0=Alu.mult, op1=Alu.add,
                        )
                        if r == 1:
                            yo = wpool.tile([P, G1], bf16, tag="yout16")
                            nc.scalar.activation(yo[:], t_sb[:], Act.Relu, scale=dcol)
                            nc.sync.dma_start(
                                out=ag_in[2][b * P:(b + 1) * P, :], in_=yo[:]
                            )
                        elif r == 2:
                            out1 = wpool.tile([P, G1], f32, tag="out1")
                            nc.scalar.activation(out1[:], t_sb[:], Act.Relu)
                            hsum = wpool.tile([P, HID_C], f32, tag="hsum")
                            nc.vector.tensor_add(
                                hsum[:], out1[:, :HID_C], out1[:, HID_C:]
                            )
                            yo = wpool.tile([P, FG], bf16, tag="yout16")
                            nc.scalar.activation(
                                yo[:, :HID_C], hsum[:], Act.Copy,
                                scale=dinvh[:, b:b + 1],
                            )
                            nc.sync.dma_start(
                                out=ag_in[3][b * P:(b + 1) * P, :], in_=yo[:]
                            )
                            hT_ps = ppool.tile([HID_C, P], f32, tag="tps")
                            nc.tensor.transpose(hT_ps[:], hsum[:], ident[:])
                            hT = wpool.tile([HID_C, P], f32, tag="aggT")
                            nc.scalar.activation(hT[:], hT_ps[:], Act.Copy)
                            r2_ps = ppool.tile([P, G2], f32, tag="mmps")
                            nc.tensor.matmul(
                                r2_ps[:], lhsT=hT[:], rhs=rootw2_s,
                                start=True, stop=True,
                            )
                            nc.vector.tensor_add(root2[:, b, :], r2_ps[:], b2rep[:])
                        elif r == 3:
                            yo = wpool.tile([P, FG], bf16, tag="yout16")
                            nc.scalar.activation(
                                yo[:, :G2], t_sb[:], Act.Relu, scale=dcol
                            )
                            nc.sync.dma_start(
                                out=ag_in[4][b * P:(b + 1) * P, :], in_=yo[:]
                            )
                        else:
                            ofin = wpool.tile([P, G2], f32, tag="out1")
                            nc.scalar.activation(ofin[:], t_sb[:], Act.Relu)
                            msum = wpool.tile([P, OUT_C], f32, tag="hsum")
                            nc.vector.tensor_add(
                                msum[:], ofin[:, :OUT_C], ofin[:, OUT_C:]
                            )
                            yo = wpool.tile([P, OUT_C], f32, tag="yout")
                            nc.scalar.activation(yo[:], msum[:], Act.Copy, scale=0.5)
                            nc.sync.dma_start(
                                out=out_d[b * P:(b + 1) * P, :], in_=yo[:]
                            )
                if r < 4 and r < max_round and not os.environ.get("GNN_SKIP_AG"):
                    nc.gpsimd.collective_compute(
                        "AllGather", Alu.bypass, replica_groups=rg,
                        ins=[ag_in[r + 1].ap().opt()], outs=[y[r + 1].ap().opt()],
                    )
                    nc.sync.dma_start(
                        out=ym[r + 1][:, :], in_=y[r + 1][:, :]
                    )

    nc.compile()
    return nc


def build_in_maps(inputs: dict, cfg: Cfg, st: Struct) -> list:
    x = np.asarray(inputs["x"], dtype=np.float32)
    wmap = build_weight_inputs(inputs, cfg)
    # host-side dinv: deg by dst, dinv = deg**-0.5 (0 where deg==0)
    dst = np.asarray(inputs["edge_index"][1], dtype=np.int64)
    deg = np.bincount(dst, minlength=cfg.N).astype(np.float64)
    dinv_n = np.where(deg > 0, deg ** -0.5, 0.0).astype(np.float32)
    in_maps = []
    for c in range(cfg.CORES):
        xs = np.zeros((cfg.NPC, cfg.IN_C), np.float32)
        dv = np.zeros(cfg.NPC, np.float32)
        mine = np.nonzero(st.pid // cfg.NPC == c)[0]
        loc = st.pid[mine] % cfg.NPC
        xs[loc] = x[mine]
        dv[loc] = dinv_n[mine]
        m = {
            "xs": xs,
            "idx16": np.ascontiguousarray(st.idx16[c]),
            # dinv[p, b] = dinv of local node b*128+p
            "dinv": np.ascontiguousarray(
                dv.reshape(cfg.blocks, P).T
            ),
        }
        m.update(wmap)
        in_maps.append(m)
    return in_maps


def assemble_output(results: list, cfg: Cfg, st: Struct) -> np.ndarray:
    full = np.concatenate(
        [np.asarray(results[c]["out"]) for c in range(cfg.CORES)], axis=0
    )
    return np.ascontiguousarray(full[st.pid]).astype(np.float32)


def kernel(**inputs) -> np.ndarray:
    from concourse.bass_utils import run_bass_kernel_spmd

    cfg = Cfg()
    st = build_structure(np.asarray(inputs["edge_index"]), cfg)
    nc = build_nc(cfg, st)
    in_maps = build_in_maps(inputs, cfg, st)
    res = run_bass_kernel_spmd(nc, in_maps, core_ids=list(range(cfg.CORES)))
    return assemble_output(res.results, cfg, st)


if __name__ == "__main__":
    pass


# revision 45
# speedup vs baseline: 1.0650x; 1.0650x over previous
"""Trainium2 Bass kernel for nn_BiARMA (2-layer ARMA GNN, K=2 stacks, T=2).

Math: A = D^-1/2 C D^-1/2 (C = edge-count matrix, deg by dst).
Key identity: norm[e] = dinv[src]*dinv[dst] factors, so
  segment_sum(out[src]*norm, dst) = dinv_dst * gather_sum(dinv_src*out[src])
-> every message-passing round is a pure row-gather-accumulate of a
pre-scaled node tensor.  Weights commute with aggregation, so matmuls
run on the aggregated tensor.

Distribution: dst-nodes sharded over 8 cores (graph parallel).  Each
core keeps a full replica of the current pre-scaled node tensor in its
DRAM, gathers rows for its local edges with the gpsimd dma_gather
ucode op, reduces padded per-node slots on DVE, applies weights on PE,
and AllGathers its updated shard each round.

Gather indices are int16 (<=32768 addressable rows), but the replica
has 50176 rows.  Two gather calls per group with OVERLAPPING windows:
call A reads rows [0, 32768) and call B reads rows [17408, 50176) of
the same replica.  Sources on cores 0-2 must use call A, cores 5-7
call B, and cores 3-4 (filled with the highest out-degree nodes) can
use either -- those flexible edges are assigned per destination to
balance the two calls, which makes the per-block padded slot counts
(max over cores x 128 partition rows) nearly tight: ~899 padded slot
columns vs 782 ideal vs 1387 for a disjoint-half split.
"""

import os
import sys
from dataclasses import dataclass, field

import numpy as np

sys.path.insert(0, "/opt/trn_rl_repo")

P = 128
WIN = 32768  # rows addressable by one int16-indexed gather call


@dataclass
class Cfg:
    N: int = 50000
    E: int = 800000
    IN_C: int = 64
    HID_C: int = 64
    OUT_C: int = 32
    K: int = 2
    CORES: int = 8
    # gather-tile budget, elements per partition per group (per dtype)
    group_budget_elems: int = int(os.environ.get("GNN_BUDGET", "6144"))

    @property
    def blocks(self):
        return (self.N // self.CORES + 1 + P - 1) // P

    @property
    def NPC(self):
        return self.blocks * P

    @property
    def NREP(self):
        return self.CORES * self.NPC

    @property
    def OFFB(self):  # window-B base row
        return self.NREP - WIN


@dataclass
class Struct:
    DA: list
    DB: list
    col_off: list       # per-block column offset (A+B combined)
    a_off: list         # per-block offset into the A column space
    b_off: list         # per-block offset into the B column space
    tot_cols: int
    idx16: np.ndarray   # [CORES, 128, (TA+TB)*8] int16 wrapped+replicated
    idx32: np.ndarray   # [CORES, P, tot_cols] int32, -1 padded (deg helper)
    pid: np.ndarray
    a_cum: list = None
    b_cum: list = None
    TA: int = 0
    TB: int = 0
    groups: dict = field(default_factory=dict)


def build_structure(edge_index: np.ndarray, cfg: Cfg) -> Struct:
    src = np.asarray(edge_index[0], dtype=np.int64)
    dst = np.asarray(edge_index[1], dtype=np.int64)
    N, CORES, NPC, NB = cfg.N, cfg.CORES, cfg.NPC, cfg.blocks
    OFFB = cfg.OFFB

    # ---- core assignment: highest out-degree nodes fill cores 3,4 (the
    # cores whose pid range lies inside BOTH gather windows), everything
    # else round-robins over the remaining cores ----
    outdeg = np.bincount(src, minlength=N)
    o = np.argsort(-outdeg, kind="stable")
    core_of = np.empty(N, np.int64)
    nflex = 2 * NPC
    core_of[o[:nflex]] = np.where(np.arange(nflex) % 2 == 0, 3, 4)
    rest_cores = np.array([0, 1, 2, 5, 6, 7])
    core_of[o[nflex:]] = rest_cores[np.arange(N - nflex) % 6]

    # ---- call assignment per edge: by source core, flexible edges
    # (src on cores 3,4) greedily balance each destination's two calls ----
    sc = core_of[src]
    a_fixed = sc <= 2
    b_fixed = sc >= 5
    cA = np.zeros(N, np.int64)
    cB = np.zeros(N, np.int64)
    np.add.at(cA, dst[a_fixed], 1)
    np.add.at(cB, dst[b_fixed], 1)
    half = np.where(a_fixed, 0, 1).astype(np.int64)
    fidx = np.nonzero(~a_fixed & ~b_fixed)[0]
    fidx = fidx[np.argsort(dst[fidx], kind="stable")]
    fd = dst[fidx]
    # per-dst greedy balance, vectorized: within each dst's flexible run,
    # first |imb| edges go to the lighter side, then alternate
    runs = np.concatenate([[0], np.cumsum(np.bincount(fd, minlength=N))])
    pos = np.arange(fidx.shape[0]) - runs[fd]
    imb = (cA - cB)[fd]  # >0: A heavier -> first flex edges go to B
    rem = pos - np.abs(imb)
    go_b = np.where(rem < 0, imb > 0, (rem % 2 == 1) ^ (imb < 0))
    half[fidx] = go_b.astype(np.int64)
    cA = np.zeros(N, np.int64)
    cB = np.zeros(N, np.int64)
    np.add.at(cA, dst[half == 0], 1)
    np.add.at(cB, dst[half == 1], 1)

    # ---- local ordering within each core: lexicographic (cA, cB) desc
    # -> tight per-block padded widths ----
    local_of = np.empty(N, np.int64)
    for c in range(CORES):
        nodes = np.nonzero(core_of == c)[0]
        o2 = np.lexsort((-cB[nodes], -cA[nodes]))
        local_of[nodes[o2]] = np.arange(len(nodes))
    pid = core_of * NPC + local_of

    # window sanity: A-call sources have pid < WIN, B-call >= OFFB
    spid = pid[src]
    assert spid[half == 0].max() < WIN
    assert spid[half == 1].min() >= OFFB

    ecore = core_of[dst]
    dloc = local_of[dst]
    sloc = np.where(half == 0, spid, spid - OFFB)  # window-local index

    # per (call, core, node) counts -> per-block padded A/B widths
    cnt = np.zeros((2, CORES, NPC), np.int64)
    for h in (0, 1):
        for c in range(CORES):
            m = (ecore == c) & (half == h)
            cnt[h, c] = np.bincount(dloc[m], minlength=NPC)
    DA = cnt[0].reshape(CORES, NB, P).max(axis=(0, 2))
    DB = cnt[1].reshape(CORES, NB, P).max(axis=(0, 2))
    DA = np.maximum(DA, 1).tolist()
    DB = np.maximum(DB, 1).tolist()
    D = [DA[b] + DB[b] for b in range(NB)]
    col_off = np.concatenate([[0], np.cumsum(D)]).tolist()
    a_off = [col_off[b] for b in range(NB)]          # A slots first per block
    b_off = [col_off[b] + DA[b] for b in range(NB)]  # then B slots
    tot_cols = int(col_off[-1])

    # per-slot values, node-major layout [P, tot_cols]
    vals = np.full((CORES, P, tot_cols), -1, np.int64)
    eo = np.lexsort((dloc, ecore))
    ecore_s, dloc_s, sloc_s, half_s = ecore[eo], dloc[eo], sloc[eo], half[eo]
    aoff = np.asarray(a_off)
    boff = np.asarray(b_off)
    for c in range(CORES):
        m = ecore_s == c
        dl, sl, hh = dloc_s[m], sloc_s[m], half_s[m]
        for h in (0, 1):
            mh = hh == h
            dlh, slh = dl[mh], sl[mh]
            cth = np.bincount(dlh, minlength=NPC)
            starts = np.concatenate([[0], np.cumsum(cth)])[:-1]
            p_in = np.arange(dlh.shape[0]) - starts[dlh]
            b = dlh // P
            p = dlh % P
            col = (aoff if h == 0 else boff)[b] + p_in
            vals[c, p, col] = slh

    # int32 deg helper (-1 = pad)
    idx32 = vals.astype(np.int32)

    st = Struct(DA=DA, DB=DB, col_off=col_off, a_off=a_off, b_off=b_off,
                tot_cols=tot_cols, idx16=None, idx32=idx32, pid=pid)

    def make_groups(budget_cols):
        budget = max(budget_cols, max(D))
        groups = []
        b0 = 0
        while b0 < NB:
            b1 = b0
            tot = 0
            while b1 < NB and (tot + D[b1] <= budget or b1 == b0):
                tot += D[b1]
                b1 += 1
            groups.append((b0, b1))
            b0 = b1
        return groups

    # every round gathers 128-bf16 rows, so group budget is per-round
    # uniform (elements per partition per group / 128 cols)
    be = cfg.group_budget_elems
    st.groups = {r: make_groups(be // 128) for r in (1, 2, 3, 4)}

    # int16 gather arrays.  For ANY contiguous block range, call A reads
    # the A-columns of blocks b0..b1 in block order, call B the
    # B-columns.  Store two wrapped arrays (all A-columns block-major,
    # then all B-columns) so every call's index slice is contiguous.
    # Flat call position i -> (partition i%128, out column i//128);
    # wrapped storage (i%16, i//16), replicated x8 to 128 partitions.
    a_cum = np.concatenate([[0], np.cumsum(DA)]).astype(np.int64)
    b_cum = np.concatenate([[0], np.cumsum(DB)]).astype(np.int64)
    TA, TB = int(a_cum[-1]), int(b_cum[-1])
    st.a_cum = a_cum.tolist()
    st.b_cum = b_cum.tolist()
    st.TA, st.TB = TA, TB

    # pad rows: phantom (always-zero) rows inside each window
    PADA = NPC - 1               # core 0, top local -- phantom
    PADB = CORES * NPC - 1 - OFFB  # core 7, top local, window-B-relative
    assert PADA < WIN and 0 <= PADB < WIN

    idx16 = np.empty((CORES, 16, (TA + TB) * 8), np.int16)
    for c in range(CORES):
        va = np.empty((P, TA), np.int64)
        vb = np.empty((P, TB), np.int64)
        for b in range(NB):
            va[:, a_cum[b]:a_cum[b + 1]] = \
                vals[c][:, a_off[b]:a_off[b] + DA[b]]
            vb[:, b_cum[b]:b_cum[b + 1]] = \
                vals[c][:, b_off[b]:b_off[b] + DB[b]]
        va = np.where(va < 0, PADA, va)
        vb = np.where(vb < 0, PADB, vb)
        both = np.concatenate([va, vb], axis=1).astype(np.int16)
        # column c', partition p -> flat i = c'*128 + p -> (i%16, i//16):
        # wrapped[r, c'*8 + q] with p = q*16 + r
        w = both.reshape(16, 8, TA + TB, order="F")
        w2 = np.transpose(w, (0, 2, 1)).reshape(16, (TA + TB) * 8)
        idx16[c] = w2
    st.idx16 = np.tile(idx16, (1, 8, 1))
    return st


# packed-weight column layout in the single [128, 768] f32 input:
# w1bd 0:128 | rootw1 128:256 | initw1 256:384 | rootw2 384:448 |
# initw2 448:512 | w2bd 512:576 | b1 576:704 (row 0) | b2 704:768 (row 0)
WPACK_COLS = 768


def build_weight_inputs(inp: dict, cfg: Cfg) -> dict:
    K, IN_C, HID_C, OUT_C = cfg.K, cfg.IN_C, cfg.HID_C, cfg.OUT_C

    rootw1 = np.transpose(inp["root_w1"][0], (1, 0, 2)).reshape(IN_C, K * HID_C)
    b1row = inp["b1"][0, :, 0, :].reshape(1, K * HID_C)
    initw1 = np.transpose(inp["init_w1"], (1, 0, 2)).reshape(IN_C, K * HID_C)
    w1bd = np.zeros((K * HID_C, K * HID_C), np.float32)
    for k in range(K):
        w1bd[k * HID_C:(k + 1) * HID_C, k * HID_C:(k + 1) * HID_C] = inp["w1"][0, k]

    # 0.5 absorbed: round-2 h-stage feeds the UNhalved stack sum into root2
    rootw2 = 0.5 * np.transpose(inp["root_w2"][0], (1, 0, 2)).reshape(HID_C, K * OUT_C)
    b2row = inp["b2"][0, :, 0, :].reshape(1, K * OUT_C)
    initw2 = np.transpose(inp["init_w2"], (1, 0, 2)).reshape(HID_C, K * OUT_C)
    w2bd = np.zeros((K * OUT_C, K * OUT_C), np.float32)
    for k in range(K):
        w2bd[k * OUT_C:(k + 1) * OUT_C, k * OUT_C:(k + 1) * OUT_C] = inp["w2"][0, k]

    wp = np.zeros((128, WPACK_COLS), np.float32)
    wp[:, 0:128] = w1bd
    wp[:IN_C, 128:256] = rootw1
    wp[:IN_C, 256:384] = initw1
    wp[:HID_C, 384:448] = rootw2
    wp[:HID_C, 448:512] = initw2
    wp[:K * OUT_C, 512:576] = w2bd
    wp[0, 576:704] = b1row[0]
    wp[0, 704:768] = b2row[0]
    return {"wpack": wp}


def build_nc(cfg: Cfg, st: Struct):
    import concourse.bacc as bacc
    import concourse.bass as bass
    import concourse.mybir as mybir
    import concourse.tile as tile
    from concourse.masks import make_identity

    f32 = mybir.dt.float32
    bf16 = mybir.dt.bfloat16
    i16 = mybir.dt.int16
    i32 = mybir.dt.int32
    X = mybir.AxisListType.X
    Alu = mybir.AluOpType
    Act = mybir.ActivationFunctionType

    K, IN_C, HID_C, OUT_C = cfg.K, cfg.IN_C, cfg.HID_C, cfg.OUT_C
    G1 = K * HID_C   # 128
    G2 = K * OUT_C   # 64
    NB = cfg.blocks
    NPC, NREP, OFFB = cfg.NPC, cfg.NREP, cfg.OFFB
    DA, DB = st.DA, st.DB
    TA, TB = st.TA, st.TB
    a_cum, b_cum = st.a_cum, st.b_cum
    WTOT = (TA + TB) * 8

    NQ = int(os.environ.get("GNN_QUEUES", "4"))
    nc = bacc.Bacc(
        "TRN2",
        target_bir_lowering=False,
        debug=False,
        num_devices=cfg.CORES,
        num_swdge_queues=NQ,
    )

    # ---- kernel I/O ----
    xs = nc.dram_tensor("xs", [NPC, IN_C], f32, kind="ExternalInput")
    idx16_d = nc.dram_tensor("idx16", [P, WTOT], i16, kind="ExternalInput")
    dinv_d = nc.dram_tensor("dinv", [P, NB], f32, kind="ExternalInput")
    wpack_d = nc.dram_tensor("wpack", [P, WPACK_COLS], f32, kind="ExternalInput")
    out_d = nc.dram_tensor("out", [NPC, OUT_C], f32, kind="ExternalOutput")

    # ---- internal DRAM: every replica row is 128 bf16 = 256B (the
    # gather's minimum descriptor payload).  Rounds whose real feature
    # width is 64 use only the lower half of each row; the upper half is
    # whatever garbage the writers leave there -- it is gathered but the
    # reduce never reads it.  bf16/elem-128 gathers measure ~2.2x faster
    # than f32/elem-64 at the same descriptor count and bytes. ----
    FG = 128  # gathered row width (bf16 elems) for every round
    y = {
        r: nc.dram_tensor(f"y{r}", [NREP, FG], bf16, addr_space="Shared")
        for r in (1, 2, 3, 4)
    }
    # gather-side mirrors in plain DRAM: random 256B reads from Shared
    # address space measure ~165us/round slower than from local DRAM, so
    # after each AllGather the replica is copied (sequential, ~50us) and
    # the gathers read the mirror.
    ym = {
        r: nc.dram_tensor(f"ym{r}", [NREP, FG], bf16)
        for r in (1, 2, 3, 4)
    }
    ag_in = {
        r: nc.dram_tensor(f"agin{r}", [NPC, FG], bf16)
        for r in (1, 2, 3, 4)
    }
    # consumed feature width per round (lower slice of each gathered row)
    FW = {1: IN_C, 2: G1, 3: HID_C, 4: G2}
    GW = {1: G1, 2: G1, 3: G2, 4: G2}
    if os.environ.get("GNN_LITE") == "5":
        ydummy = nc.dram_tensor("ydummy", [NREP, FG], bf16)

    rg = [list(range(cfg.CORES))]

    max_gt = max(
        max((st.col_off[b1] - st.col_off[b0]) * FG
            for (b0, b1) in st.groups[r])
        for r in (1, 2, 3, 4)
    )

    NSEM = 8
    dsems = [nc.alloc_semaphore(f"gsem{i}") for i in range(NSEM)]
    sem_count = [0] * NSEM
    gidx = [0]  # global gather-call counter
    wait_a = [None]
    wait_b = [None]

    with tile.TileContext(nc) as tc:
        with (
            tc.tile_pool(name="const", bufs=1) as cpool,
            tc.tile_pool(name="gather", bufs=int(os.environ.get("GNN_GBUFS", "5"))) as gpool,
            tc.tile_pool(name="work", bufs=3) as wpool,
            tc.tile_pool(name="psum", bufs=3, space="PSUM") as ppool,
        ):
            # ---------- constants ----------
            ident = cpool.tile([P, P], f32)
            make_identity(nc, ident[:])

            wsb = cpool.tile([P, WPACK_COLS], f32)
            nc.sync.dma_start(out=wsb[:], in_=wpack_d[:, :])
            w1bd_s = wsb[:, 0:128]
            rootw1_s = wsb[0:IN_C, 128:256]
            initw1_s = wsb[0:IN_C, 256:384]
            rootw2_s = wsb[0:HID_C, 384:448]
            initw2_s = wsb[0:HID_C, 448:512]
            w2bd_s = wsb[0:G2, 512:576]
            b1_s = wsb[0:1, 576:704]
            b2_s = wsb[0:1, 704:768]
            rhs_s = {1: initw1_s, 2: w1bd_s, 3: initw2_s, 4: w2bd_s}

            ones1 = cpool.tile([1, P], f32)
            nc.vector.memset(ones1[:], 1.0)
            b1rep = cpool.tile([P, G1], f32)
            b2rep = cpool.tile([P, G2], f32)
            bps = ppool.tile([P, G1], f32, tag="mmps")
            nc.tensor.matmul(bps[:], lhsT=ones1[:], rhs=b1_s, start=True, stop=True)
            nc.vector.tensor_copy(b1rep[:], bps[:])
            bps2 = ppool.tile([P, G2], f32, tag="mmps")
            nc.tensor.matmul(bps2[:], lhsT=ones1[:], rhs=b2_s, start=True, stop=True)
            nc.vector.tensor_copy(b2rep[:], bps2[:])

            # ---------- gather indices + degrees ----------
            idx16_s = cpool.tile([P, WTOT], i16)
            nc.sync.dma_start(out=idx16_s[:], in_=idx16_d[:, :])

            root1 = cpool.tile([P, NB, G1], f32)
            root2 = cpool.tile([P, NB, G2], f32)
            dinv = cpool.tile([P, NB], f32)
            dinvh = cpool.tile([P, NB], f32)
            nc.sync.dma_start(out=dinv[:], in_=dinv_d[:, :])
            nc.vector.tensor_scalar_mul(dinvh[:], dinv[:], 0.5)
            with tc.tile_pool(name="prolog", bufs=1) as qpool:
                # ---------- roots + Y1 ----------
                x_s = qpool.tile([P, NB, IN_C], f32)
                for b in range(NB):
                    nc.sync.dma_start(
                        out=x_s[:, b, :], in_=xs[b * P:(b + 1) * P, :]
                    )
                for b in range(NB):
                    dcol = dinv[:, b:b + 1]
                    xT_ps = ppool.tile([IN_C, P], f32, tag="tps")
                    nc.tensor.transpose(xT_ps[:], x_s[:, b, :], ident[:])
                    xT = wpool.tile([IN_C, P], f32, tag="aggT")
                    nc.scalar.activation(xT[:], xT_ps[:], Act.Copy)
                    r1_ps = ppool.tile([P, G1], f32, tag="mmps")
                    nc.tensor.matmul(
                        r1_ps[:], lhsT=xT[:], rhs=rootw1_s, start=True, stop=True
                    )
                    nc.vector.tensor_add(root1[:, b, :], r1_ps[:], b1rep[:])
                    y1b = wpool.tile([P, 128], bf16, tag="yout16")
                    nc.scalar.activation(
                        y1b[:, :IN_C], x_s[:, b, :], Act.Copy, scale=dcol
                    )
                    nc.sync.dma_start(
                        out=ag_in[1][b * P:(b + 1) * P, :], in_=y1b[:]
                    )
            if not os.environ.get("GNN_SKIP_AG0"):
                nc.gpsimd.collective_compute(
                    "AllGather", Alu.bypass, replica_groups=rg,
                    ins=[ag_in[1].ap().opt()], outs=[y[1].ap().opt()],
                )
                nc.sync.dma_start(out=ym[1][:, :], in_=y[1][:, :])

            # ---------- 4 message-passing rounds ----------
            max_round = int(os.environ.get("GNN_STAGE", "4"))
            reps = int(os.environ.get("GNN_REPS", "1"))

            # first groups of each round read the Shared tensor directly
            # (ready right after the AllGather) so the mirror copy
            # overlaps them instead of delaying the round start
            YDIRECT = int(os.environ.get("GNN_YDIRECT", "5"))

            def gather_group(r, b0, b1, gi=99):
                """Issue the A and B dma_gather calls for blocks [b0,b1);
                returns the gather tile (cols: A of b0..b1, then B)."""
                nA = (a_cum[b1] - a_cum[b0]) * P
                nB = (b_cum[b1] - b_cum[b0]) * P
                ncols = (nA + nB) // P
                gt = gpool.tile([P, max_gt], bf16, tag="gt")
                outA = gt[:, :nA // P * FG].rearrange("p (c f) -> p c f", f=FG)
                outB = gt[:, nA // P * FG:ncols * FG].rearrange(
                    "p (c f) -> p c f", f=FG
                )
                ixA = idx16_s[:, a_cum[b0] * 8:a_cum[b1] * 8]
                ixB = idx16_s[:, (TA + b_cum[b0]) * 8:(TA + b_cum[b1]) * 8]
                if os.environ.get("GNN_LITE") == "5":
                    yv = ydummy
                else:
                    yv = y[r] if gi < YDIRECT else ym[r]
                sA = (2 * gidx[0]) % NSEM
                sB = (2 * gidx[0] + 1) % NSEM
                qA = (2 * gidx[0]) % NQ
                qB = (2 * gidx[0] + 1) % NQ
                gidx[0] += 1
                sem_count[sA] += 16
                sem_count[sB] += 16
                nc.gpsimd.dma_gather(
                    out_ap=outA, in_ap=yv[0:WIN, :], idxs_ap=ixA,
                    num_idxs=nA, num_idxs_reg=nA, elem_size=FG,
                    single_packet=False, queue_num=qA,
                ).then_inc(dsems[sA], 16)
                nc.gpsimd.dma_gather(
                    out_ap=outB, in_ap=yv[OFFB:NREP, :], idxs_ap=ixB,
                    num_idxs=nB, num_idxs_reg=nB, elem_size=FG,
                    single_packet=False, queue_num=qB,
                ).then_inc(dsems[sB], 16)
                wait_a[0] = (dsems[sA], sem_count[sA])
                wait_b[0] = (dsems[sB], sem_count[sB])
                return gt, nA // P

            lite = int(os.environ.get("GNN_LITE", "0"))
            for rep in range(reps):
              for r in (1, 2, 3, 4):
                if r > max_round:
                    break
                F = FW[r]
                G = GW[r]
                for gi, (b0, b1) in enumerate(st.groups[r]):
                    gt, colsA = gather_group(r, b0, b1, gi)
                    if lite == 6:
                        # microbench-style: Pool-side flow control only
                        if gidx[0] > 3:
                            gpast = gidx[0] - 4
                            sa = (2 * gpast) % NSEM
                            nc.gpsimd.wait_ge(
                                dsems[sa], 16 * (gpast // (NSEM // 2) + 1)
                            )
                        continue
                    if lite in (1, 4, 5):
                        lt = wpool.tile([P, 2 * FG], f32, tag="lt")
                        cpa = nc.vector.tensor_copy(lt[:, :FG], gt[:, :FG])
                        cpb = nc.vector.tensor_copy(
                            lt[:, FG:], gt[:, colsA * FG:(colsA + 1) * FG]
                        )
                        if lite == 1:
                            cpa._wait_ge(*wait_a[0])
                            cpb._wait_ge(*wait_b[0])
                        continue
                    for b in range(b0, b1):
                        dcol = dinv[:, b:b + 1]
                        oA = a_cum[b] - a_cum[b0]
                        oB = colsA + (b_cum[b] - b_cum[b0])
                        aggA = wpool.tile([P, F], f32, tag="aggA")
                        rA = nc.vector.reduce_sum(
                            aggA[:],
                            gt[:, oA * FG:(oA + DA[b]) * FG].rearrange(
                                "p (d f) -> p f d", f=FG
                            )[:, 0:F, :],
                            axis=X,
                        )
                        aggB = wpool.tile([P, F], f32, tag="aggB")
                        rB = nc.vector.reduce_sum(
                            aggB[:],
                            gt[:, oB * FG:(oB + DB[b]) * FG].rearrange(
                                "p (d f) -> p f d", f=FG
                            )[:, 0:F, :],
                            axis=X,
                        )
                        rA._wait_ge(*wait_a[0])
                        rB._wait_ge(*wait_b[0])
                        if lite == 2:
                            continue
                        agg = wpool.tile([P, F], f32, tag="agg")
                        nc.vector.tensor_add(agg[:], aggA[:], aggB[:])
                        aggT_ps = ppool.tile([F, P], f32, tag="tps")
                        nc.tensor.transpose(aggT_ps[:], agg[:], ident[:])
                        aggT = wpool.tile([F, P], f32, tag="aggT")
                        nc.scalar.activation(aggT[:], aggT_ps[:], Act.Copy)
                        mm_ps = ppool.tile([P, G], f32, tag="mmps")
                        nc.tensor.matmul(
                            mm_ps[:], lhsT=aggT[:], rhs=rhs_s[r],
                            start=True, stop=True,
                        )
                        if lite == 3:
                            continue
                        root = root1 if r <= 2 else root2
                        t_sb = wpool.tile([P, G], f32, tag="tsb")
                        nc.vector.scalar_tensor_tensor(
                            t_sb[:], mm_ps[:], dcol, root[:, b, :],
                            op0=Alu.mult, op1=Alu.add,
                        )
                        if r == 1:
                            yo = wpool.tile([P, G1], bf16, tag="yout16")
                            nc.scalar.activation(yo[:], t_sb[:], Act.Relu, scale=dcol)
                            nc.sync.dma_start(
                                out=ag_in[2][b * P:(b + 1) * P, :], in_=yo[:]
                            )
                        elif r == 2:
                            out1 = wpool.tile([P, G1], f32, tag="out1")
                            nc.scalar.activation(out1[:], t_sb[:], Act.Relu)
                            hsum = wpool.tile([P, HID_C], f32, tag="hsum")
                            nc.vector.tensor_add(
                                hsum[:], out1[:, :HID_C], out1[:, HID_C:]
                            )
                            yo = wpool.tile([P, FG], bf16, tag="yout16")
                            nc.scalar.activation(
                                yo[:, :HID_C], hsum[:], Act.Copy,
                                scale=dinvh[:, b:b + 1],
                            )
                            nc.sync.dma_start(
                                out=ag_in[3][b * P:(b + 1) * P, :], in_=yo[:]
                            )
                            hT_ps = ppool.tile([HID_C, P], f32, tag="tps")
                            nc.tensor.transpose(hT_ps[:], hsum[:], ident[:])
                            hT = wpool.tile([HID_C, P], f32, tag="aggT")
                            nc.scalar.activation(hT[:], hT_ps[:], Act.Copy)
                            r2_ps = ppool.tile([P, G2], f32, tag="mmps")
                            nc.tensor.matmul(
                                r2_ps[:], lhsT=hT[:], rhs=rootw2_s,
                                start=True, stop=True,
                            )
                            nc.vector.tensor_add(root2[:, b, :], r2_ps[:], b2rep[:])
                        elif r == 3:
                            yo = wpool.tile([P, FG], bf16, tag="yout16")
                            nc.scalar.activation(
                                yo[:, :G2], t_sb[:], Act.Relu, scale=dcol
                            )
                            nc.sync.dma_start(
                                out=ag_in[4][b * P:(b + 1) * P, :], in_=yo[:]
                            )
                        else:
                            ofin = wpool.tile([P, G2], f32, tag="out1")
                            nc.scalar.activation(ofin[:], t_sb[:], Act.Relu)
                            msum = wpool.tile([P, OUT_C], f32, tag="hsum")
                            nc.vector.tensor_add(
                                msum[:], ofin[:, :OUT_C], ofin[:, OUT_C:]
                            )
                            yo = wpool.tile([P, OUT_C], f32, tag="yout")
                            nc.scalar.activation(yo[:], msum[:], Act.Copy, scale=0.5)
                            nc.sync.dma_start(
                                out=out_d[b * P:(b + 1) * P, :], in_=yo[:]
                            )
                if r < 4 and r < max_round and not os.environ.get("GNN_SKIP_AG"):
                    nc.gpsimd.collective_compute(
                        "AllGather", Alu.bypass, replica_groups=rg,
                        ins=[ag_in[r + 1].ap().opt()], outs=[y[r + 1].ap().opt()],
                    )
                    nc.sync.dma_start(
                        out=ym[r + 1][:, :], in_=y[r + 1][:, :]
                    )

    nc.compile()
    return nc


def build_in_maps(inputs: dict, cfg: Cfg, st: Struct) -> list:
    x = np.asarray(inputs["x"], dtype=np.float32)
    wmap = build_weight_inputs(inputs, cfg)
    # host-side dinv: deg by dst, dinv = deg**-0.5 (0 where deg==0)
    dst = np.asarray(inputs["edge_index"][1], dtype=np.int64)
    deg = np.bincount(dst, minlength=cfg.N).astype(np.float64)
    dinv_n = np.where(deg > 0, deg ** -0.5, 0.0).astype(np.float32)
    in_maps = []
    for c in range(cfg.CORES):
        xs = np.zeros((cfg.NPC, cfg.IN_C), np.float32)
        dv = np.zeros(cfg.NPC, np.float32)
        mine = np.nonzero(st.pid // cfg.NPC == c)[0]
        loc = st.pid[mine] % cfg.NPC
        xs[loc] = x[mine]
        dv[loc] = dinv_n[mine]
        m = {
            "xs": xs,
            "idx16": np.ascontiguousarray(st.idx16[c]),
            # dinv[p, b] = dinv of local node b*128+p
            "dinv": np.ascontiguousarray(
                dv.reshape(cfg.blocks, P).T
            ),
        }
        m.update(wmap)
        in_maps.append(m)
    return in_maps


def assemble_output(results: list, cfg: Cfg, st: Struct) -> np.ndarray:
    full = np.concatenate(
        [np.asarray(results[c]["out"]) for c in range(cfg.CORES)], axis=0
    )
    return np.ascontiguousarray(full[st.pid]).astype(np.float32)


def kernel(**inputs) -> np.ndarray:
    from concourse.bass_utils import run_bass_kernel_spmd

    cfg = Cfg()
    st = build_structure(np.asarray(inputs["edge_index"]), cfg)
    nc = build_nc(cfg, st)
    in_maps = build_in_maps(inputs, cfg, st)
    res = run_bass_kernel_spmd(nc, in_maps, core_ids=list(range(cfg.CORES)))
    return assemble_output(res.results, cfg, st)


if __name__ == "__main__":
    pass
